# revision 5
# baseline (speedup 1.0000x reference)
"""RBF-kernel SVM decision function on 8 TRN2 NeuronCores.

out[i] = sum_j alphas[j] * exp(-GAMMA * ||x[i] - supports[j]||^2)

Factorization: out_i = u_i * sum_j sgn_j * exp(e_ij),
  e_ij = (x_i/32).s_j + (ln|a_j| - g|s_j|^2)   [PSUM via one fp8 DoubleRow
         matmul; see row scheme below]
  u_i  = exp(-g|x_i|^2)                        [applied at the end, per i]

PE: bf16 matmuls stream 512-col chunks at ~427ns on this silicon; fp8
streams at ~216ns. To keep fp8's speed without its 3-bit-mantissa noise,
the exponent is built from range-scaled e4m3 hi/lo splits contracted in
one DoubleRow matmul ([128 partitions, 2 sub-rows], 196 live rows):
  A1=e4m3(x/4), A2=e4m3(16(x/4-A1)); B1=e4m3(s/8), B2=e4m3(16(s/8-B1))
  w = A1.B1 + (A1/16).B2 + A2.(B1/16)      (al*bl term dropped)
  jterm via 3 rows: J1 + J2/16 + J3/256
Measured exponent error ~6e-4 rms -- negligible vs the DVE approximation.

Both ScalarE (ACT) and the DVE drain PSUM in parallel, each doing the
complete exp+reduce job for its own j-range:
  - ACT, j in [0, A_ACT): native ACTIVATE(Exp, accum_out) in place on
    PSUM; pieces are sign-pure by construction of the host permutation
    (2048 largest-|a| positives first, then largest negatives).
  - DVE, j in [A_ACT, M): Schraudolph exp. One TENSOR_SCALAR computes
    round(1024*(log2e*e + 23 - SIGMA)) from PSUM fp32 into a uint16
    tile; the codes reinterpreted as fp16 ARE ~exp(e)*2^8 (pw-linear
    2^t, ~1.8% rms). Negative-alpha pieces use B+32768 (sign bit), so
    one TENSOR_SCALAR_CACHE_REDUCE (scale 2^-8) over the bitcast codes
    yields the signed sum directly. The DVE range gets the smallest-|a|
    supports so its error is weighted by the least-important columns.
Final per-tile combine (accP - accN1 - accN2 + dveS) * u runs on the
otherwise idle Pool (GPSIMD) engine.
"""

import os
import sys

for p in ("/opt/trn_rl_repo",):
    if p not in sys.path:
        sys.path.insert(0, p)

import numpy as np
import ml_dtypes

import concourse.bass as bass
import concourse.tile as tile
from concourse import bacc, mybir
from concourse.bass_utils import run_bass_kernel_spmd

N_CORES = 8
N = 16384
M = 8192
F = 64
GAMMA = 1.0 / F
N_LOC = N // N_CORES        # 2048 queries per core
N_TILES = N_LOC // 128      # 16 i-tiles of 128 queries
W = 2048                    # j-window: 4 PSUM banks
NW = M // W                 # 4 windows per j sweep
MM_N = 512                  # matmul moving free dim (1 PSUM bank)
NROW = 196                  # live contraction rows (of 256)

A_ACT = 5504                # ACT engine j-columns per sweep (rest -> DVE)
D_DVE = M - A_ACT
P_ACT = 2048                # positives in ACT range (= window 0 exactly)

# Schraudolph constants (fp16 code format): v = round(1024*(log2e*e + C -
# SIGMA)); the uint16 pattern read as fp16 is ~exp(e)*2^(C-15). SIGMA
# tuned for zero mean error under round-to-nearest (HW-verified rint).
SIGMA = float(os.environ.get("BASS_SIGMA", "0.0575"))
C16 = 23.0
A_SC = 1024.0 * np.log2(np.e)
B_SC = 1024.0 * (C16 - SIGMA)
CR_SCALE = 2.0 ** (15 - C16)

BF16 = mybir.dt.bfloat16
FP16 = mybir.dt.float16
F32 = mybir.dt.float32
U16 = mybir.dt.uint16
FP8 = mybir.dt.float8e4
bf16 = ml_dtypes.bfloat16
f8 = ml_dtypes.float8_e4m3fn

_compiled_cache = {}


def _build_common(nc, tc, cpool):
    """Input DRAM tensors, exp-table warmer, and input DMAs ordered so the
    first windows' operands land first."""
    x8_d = nc.dram_tensor("x8", [128, 2, N_LOC], FP8, kind="ExternalInput")
    s8_d = nc.dram_tensor("s8", [128, 2, M], FP8, kind="ExternalInput")
    u_d = nc.dram_tensor("u", [128, N_TILES], F32, kind="ExternalInput")
    out_d = nc.dram_tensor("out", [128, N_TILES], F32, kind="ExternalOutput")

    warm_act = cpool.tile([128, 1], F32)
    nc.gpsimd.memset(warm_act[:], 0.0)
    nc.scalar.activation(warm_act[:], warm_act[:], mybir.ActivationFunctionType.Exp)

    s8_sb = cpool.tile([128, 2, M], FP8)
    nc.sync.dma_start(s8_sb[:, :, 0:W], s8_d.ap()[:, :, 0:W])
    x8_sb = cpool.tile([128, 2, N_LOC], FP8)
    nc.sync.dma_start(x8_sb[:, :, 0:128], x8_d.ap()[:, :, 0:128])
    u_sb = cpool.tile([128, N_TILES], F32)
    nc.sync.dma_start(u_sb[:], u_d.ap()[:])
    for w in range(1, NW):
        nc.sync.dma_start(
            s8_sb[:, :, w * W : (w + 1) * W],
            s8_d.ap()[:, :, w * W : (w + 1) * W],
        )
    nc.sync.dma_start(x8_sb[:, :, 128:], x8_d.ap()[:, :, 128:])
    return x8_sb, s8_sb, u_sb, out_d


def _mm_window(nc, t, ps_tile, w, x8_sb, s8_sb):
    DR = mybir.MatmulPerfMode.DoubleRow
    for c in range(W // MM_N):
        nc.tensor.matmul(
            ps_tile[:, c * MM_N : (c + 1) * MM_N],
            x8_sb[:, :, t * 128 : (t + 1) * 128],
            s8_sb[:, :, w * W + c * MM_N : w * W + (c + 1) * MM_N],
            start=True,
            stop=True,
            perf_mode=DR,
        )


def _dve_pieces(cN):
    """Split the DVE range [A_ACT, M) at window edges and at cN (the
    negative->positive boundary). Returns [(col_lo, col_hi, flip)]."""
    cuts = sorted({A_ACT, M, cN} | {w * W for w in range(NW + 1) if A_ACT < w * W < M})
    pieces = []
    for lo, hi in zip(cuts[:-1], cuts[1:]):
        if lo >= A_ACT and hi <= M and lo < hi:
            pieces.append((lo, hi, hi <= cN))
    return pieces


def _build_v3(cN):
    nc = bacc.Bacc(
        "TRN2",
        target_bir_lowering=False,
        debug=False,
        enable_asserts=False,
        num_devices=N_CORES,
    )
    Exp = mybir.ActivationFunctionType.Exp
    mult = mybir.AluOpType.mult
    add = mybir.AluOpType.add
    subtract = mybir.AluOpType.subtract
    pieces = _dve_pieces(cN)

    with tile.TileContext(nc) as tc:
        with (
            tc.tile_pool(name="const", bufs=1) as cpool,
            tc.tile_pool(name="acc", bufs=4) as apool,
            tc.tile_pool(name="stg", bufs=4) as spool,
            tc.tile_pool(name="fin", bufs=4) as fpool,
            tc.tile_pool(name="psum", bufs=2, space="PSUM") as ppool,
        ):
            x8_sb, s8_sb, u_sb, out_d = _build_common(nc, tc, cpool)
            outT_sb = cpool.tile([128, N_TILES], F32)
            dvout = cpool.tile([128, D_DVE], FP16)
            # ACT exp output goes to a throwaway SBUF buffer (nobody reads
            # it): an in-place PSUM write would create a false write-vs-read
            # ordering against the DVE's read of the same PSUM tile.
            trash = cpool.tile([128, W], FP16)

            # CR pieces: split the code reduce at window edges so each part
            # can fire as soon as its TS codes land (packs DVE idle gaps).
            cr_parts = []
            for wb in range(NW):
                lo = max(A_ACT, wb * W) - A_ACT
                hi = (wb + 1) * W - A_ACT
                if hi > lo >= 0:
                    cr_parts.append((lo, hi))

            for t in range(N_TILES):
                acc = apool.tile([128, 6], F32, tag="acc")
                stg = spool.tile([128, D_DVE], U16, tag="stg")
                ncr = 0
                for w in range(NW):
                    ps_tile = ppool.tile([128, W], F32, tag="E")
                    _mm_window(nc, t, ps_tile, w, x8_sb, s8_sb)
                    wlo, whi = w * W, (w + 1) * W
                    # ACT part of this window
                    alo, ahi = wlo, min(whi, A_ACT)
                    if alo < ahi:
                        if ahi <= P_ACT:
                            acol = acc[:, 0:1]      # positives
                        elif alo < P_ACT:
                            raise AssertionError("P/N boundary must align to window edge")
                        else:
                            acol = acc[:, 1:2] if w == 1 else acc[:, 2:3]
                        nc.scalar.activation(
                            trash[:, 0 : ahi - alo],
                            ps_tile[:, alo - wlo : ahi - wlo],
                            Exp,
                            accum_out=acol,
                        )
                    # DVE (Schraudolph) part of this window
                    for plo, phi, flip in pieces:
                        if plo >= whi or phi <= wlo:
                            continue
                        nc.vector.tensor_scalar(
                            stg[:, plo - A_ACT : phi - A_ACT],
                            ps_tile[:, plo - wlo : phi - wlo],
                            A_SC,
                            B_SC + (32768.0 if flip else 0.0),
                            mult,
                            add,
                        )
                    # signed, 2^8-scaled partial sums of finished code spans.
                    # Col layout [P, N1, N2, dve1, pad0, dve2]: pairwise
                    # (even - odd) then f0[0]-f0[1]-f0[2] yields
                    # P - N1 - N2 + dve1 + dve2.
                    while ncr < len(cr_parts) and cr_parts[ncr][1] <= whi - A_ACT:
                        lo, hi = cr_parts[ncr]
                        col = (3, 5)[ncr]
                        nc.vector.tensor_scalar(
                            dvout[:, lo:hi],
                            stg[:, lo:hi].bitcast(FP16),
                            CR_SCALE,
                            0.0,
                            mult,
                            add,
                            accum_out=acc[:, col : col + 1],
                        )
                        ncr += 1
                assert ncr == 2, ncr
                nc.gpsimd.memset(acc[:, 4:5], 0.0)
                # out = u * ((P-N1) - (N2-dve1) - (0?-dve2)) pattern:
                # f0 = [P-N1, N2-dve1, pad-dve2]; out = u*(f0[0]-f0[1]-f0[2])
                f0 = fpool.tile([128, 3], F32, tag="fin")
                nc.gpsimd.tensor_tensor(f0[:], acc[:, 0:6:2], acc[:, 1:6:2], subtract)
                f1 = fpool.tile([128, 1], F32, tag="fin2")
                nc.gpsimd.tensor_tensor(f1[:], f0[:, 0:1], f0[:, 1:2], subtract)
                f2 = fpool.tile([128, 1], F32, tag="fin3")
                nc.gpsimd.tensor_tensor(f2[:], f1[:], f0[:, 2:3], subtract)
                nc.gpsimd.tensor_tensor(
                    outT_sb[:, t : t + 1], f2[:], u_sb[:, t : t + 1], mult
                )

            nc.sync.dma_start(out_d.ap()[:], outT_sb[:])

    nc.compile()
    return nc


def _f8(v):
    return v.astype(f8)


def _prepare(x, supports, alphas):
    x = np.asarray(x, dtype=np.float32)
    supports = np.asarray(supports, dtype=np.float32)
    alphas = np.asarray(alphas, dtype=np.float32)

    a64 = alphas.astype(np.float64)
    s64 = supports.astype(np.float64)
    jterm = -GAMMA * (s64 * s64).sum(axis=1) + np.maximum(
        np.log(np.maximum(np.abs(a64), 1e-300)), -11.0
    )

    order = np.argsort(np.abs(a64), kind="stable")
    allP = order[a64[order] > 0]
    allN = order[a64[order] <= 0]
    n_act_N = A_ACT - P_ACT
    assert len(allP) >= P_ACT and len(allN) >= n_act_N, (len(allP), len(allN))
    act_P = allP[-P_ACT:]
    dve_P = allP[:-P_ACT]
    act_N = allN[-n_act_N:]
    dve_N = allN[:-n_act_N]
    perm = np.concatenate([act_P, act_N, dve_N, dve_P])
    cN = A_ACT + len(dve_N)  # DVE cols in [A_ACT, cN) are negative-alpha

    # fp8 range-scaled hi/lo splits
    xs4 = (x.T / 4.0).astype(np.float64)              # [F, N]
    sp8 = (supports[perm].T / 8.0).astype(np.float64)  # [F, M]
    A1 = _f8(xs4)
    A2 = _f8(16.0 * (xs4 - A1.astype(np.float64)))
    A1o16 = _f8(A1.astype(np.float64) / 16.0)
    B1 = _f8(sp8)
    B2 = _f8(16.0 * (sp8 - B1.astype(np.float64)))
    B1o16 = _f8(B1.astype(np.float64) / 16.0)
    jt = jterm[perm]
    J1 = _f8(jt)
    J2 = _f8(16.0 * (jt - J1.astype(np.float64)))
    J3 = _f8(256.0 * (jt - J1.astype(np.float64) - J2.astype(np.float64) / 16.0))

    # pack logical rows r -> (p = r//2, i = r%2)
    xrows = np.zeros((256, N), dtype=f8)
    srows = np.zeros((256, M), dtype=f8)
    xrows[0:64] = A1
    srows[0:64] = B1
    xrows[64:128] = A1o16
    srows[64:128] = B2
    xrows[128:192] = A2
    srows[128:192] = B1o16
    xrows[192] = f8(1.0)
    srows[192] = J1
    xrows[193] = f8(0.0625)
    srows[193] = J2
    xrows[194] = f8(0.00390625)
    srows[194] = J3
    x8 = xrows.reshape(128, 2, N)
    s8 = srows.reshape(128, 2, M)

    u = np.exp(-GAMMA * (x.astype(np.float64) ** 2).sum(axis=1)).astype(np.float32)

    in_maps = []
    for c in range(N_CORES):
        sl = slice(c * N_LOC, (c + 1) * N_LOC)
        in_maps.append(
            {
                "x8": np.ascontiguousarray(x8[:, :, sl]),
                "s8": s8,
                "u": np.ascontiguousarray(u[sl].reshape(N_TILES, 128).T),
            }
        )
    return cN, in_maps


def _run(x, supports, alphas, trace=False, **run_kwargs):
    cN, in_maps = _prepare(x, supports, alphas)
    key = (cN, A_ACT, SIGMA)
    if key not in _compiled_cache:
        _compiled_cache[key] = _build_v3(cN)
    nc = _compiled_cache[key]
    res = run_bass_kernel_spmd(
        nc, in_maps, core_ids=list(range(N_CORES)), trace=trace, **run_kwargs
    )
    outs = [r["out"].T.reshape(-1) for r in res.results]
    return np.concatenate(outs).astype(np.float32), res


def kernel(x, supports, alphas):
    out, _ = _run(x, supports, alphas, trace=False)
    return out


# revision 7
# speedup vs baseline: 1.0431x; 1.0431x over previous
"""RBF-kernel SVM decision function on 8 TRN2 NeuronCores.

out[i] = sum_j alphas[j] * exp(-GAMMA * ||x[i] - supports[j]||^2)

Factorization: out_i = u_i * sum_j sgn_j * exp(e_ij),
  e_ij = (x_i/32).s_j + (ln|a_j| - g|s_j|^2)   [PSUM via one fp8 DoubleRow
         matmul; see row scheme below]
  u_i  = exp(-g|x_i|^2)                        [applied at the end, per i]

PE: bf16 matmuls stream 512-col chunks at ~427ns on this silicon; fp8
streams at ~216ns. To keep fp8's speed without its 3-bit-mantissa noise,
the exponent is built from range-scaled e4m3 hi/lo splits contracted in
one DoubleRow matmul ([128 partitions, 2 sub-rows], 196 live rows):
  A1=e4m3(x/4), A2=e4m3(16(x/4-A1)); B1=e4m3(s/8), B2=e4m3(16(s/8-B1))
  w = A1.B1 + (A1/16).B2 + A2.(B1/16)      (al*bl term dropped)
  jterm via 3 rows: J1 + J2/16 + J3/256
Measured exponent error ~6e-4 rms -- negligible vs the DVE approximation.

Both ScalarE (ACT) and the DVE drain PSUM in parallel, each doing the
complete exp+reduce job for its own j-range:
  - ACT, j in [0, A_ACT): native ACTIVATE(Exp, accum_out) in place on
    PSUM; pieces are sign-pure by construction of the host permutation
    (2048 largest-|a| positives first, then largest negatives).
  - DVE, j in [A_ACT, M): Schraudolph exp. One TENSOR_SCALAR computes
    round(1024*(log2e*e + 23 - SIGMA)) from PSUM fp32 into a uint16
    tile; the codes reinterpreted as fp16 ARE ~exp(e)*2^8 (pw-linear
    2^t, ~1.8% rms). Negative-alpha pieces use B+32768 (sign bit), so
    one TENSOR_SCALAR_CACHE_REDUCE (scale 2^-8) over the bitcast codes
    yields the signed sum directly. The DVE range gets the smallest-|a|
    supports so its error is weighted by the least-important columns.
Final per-tile combine (accP - accN1 - accN2 + dveS) * u runs on the
otherwise idle Pool (GPSIMD) engine.
"""

import os
import sys

for p in ("/opt/trn_rl_repo",):
    if p not in sys.path:
        sys.path.insert(0, p)

import numpy as np
import ml_dtypes

import concourse.bass as bass
import concourse.tile as tile
from concourse import bacc, mybir
from concourse.bass_utils import run_bass_kernel_spmd

N_CORES = 8
N = 16384
M = 8192
F = 64
GAMMA = 1.0 / F
N_LOC = N // N_CORES        # 2048 queries per core
N_TILES = N_LOC // 128      # 16 i-tiles of 128 queries
W = 2048                    # j-window: 4 PSUM banks
NW = M // W                 # 4 windows per j sweep
MM_N = 512                  # matmul moving free dim (1 PSUM bank)
NROW = 196                  # live contraction rows (of 256)

A_ACT = 5760                # ACT engine j-columns per sweep (rest -> DVE)
D_DVE = M - A_ACT
P_ACT = 2048                # positives in ACT range (= window 0 exactly)

# Schraudolph constants (fp16 code format): v = round(1024*(log2e*e + C -
# SIGMA)); the uint16 pattern read as fp16 is ~exp(e)*2^(C-15). SIGMA
# tuned for zero mean error under round-to-nearest (HW-verified rint).
SIGMA = float(os.environ.get("BASS_SIGMA", "0.0575"))
C16 = 23.0
A_SC = 1024.0 * np.log2(np.e)
B_SC = 1024.0 * (C16 - SIGMA)
CR_SCALE = 2.0 ** (15 - C16)

BF16 = mybir.dt.bfloat16
FP16 = mybir.dt.float16
F32 = mybir.dt.float32
U16 = mybir.dt.uint16
FP8 = mybir.dt.float8e4
bf16 = ml_dtypes.bfloat16
f8 = ml_dtypes.float8_e4m3fn

_compiled_cache = {}


def _build_common(nc, tc, cpool):
    """Input DRAM tensors, exp-table warmer, and input DMAs ordered so the
    first windows' operands land first."""
    x8_d = nc.dram_tensor("x8", [128, 2, N_LOC], FP8, kind="ExternalInput")
    s8_d = nc.dram_tensor("s8", [128, 2, M], FP8, kind="ExternalInput")
    u_d = nc.dram_tensor("u", [128, N_TILES], F32, kind="ExternalInput")
    out_d = nc.dram_tensor("out", [128, N_TILES], F32, kind="ExternalOutput")

    warm_act = cpool.tile([128, 1], F32)
    nc.gpsimd.memset(warm_act[:], 0.0)
    nc.scalar.activation(warm_act[:], warm_act[:], mybir.ActivationFunctionType.Exp)

    s8_sb = cpool.tile([128, 2, M], FP8)
    nc.sync.dma_start(s8_sb[:, :, 0:W], s8_d.ap()[:, :, 0:W])
    x8_sb = cpool.tile([128, 2, N_LOC], FP8)
    nc.sync.dma_start(x8_sb[:, :, 0:128], x8_d.ap()[:, :, 0:128])
    u_sb = cpool.tile([128, N_TILES], F32)
    nc.sync.dma_start(u_sb[:], u_d.ap()[:])
    for w in range(1, NW):
        nc.sync.dma_start(
            s8_sb[:, :, w * W : (w + 1) * W],
            s8_d.ap()[:, :, w * W : (w + 1) * W],
        )
    nc.sync.dma_start(x8_sb[:, :, 128:], x8_d.ap()[:, :, 128:])
    return x8_sb, s8_sb, u_sb, out_d


def _mm_window(nc, t, ps_tile, w, x8_sb, s8_sb):
    DR = mybir.MatmulPerfMode.DoubleRow
    for c in range(W // MM_N):
        nc.tensor.matmul(
            ps_tile[:, c * MM_N : (c + 1) * MM_N],
            x8_sb[:, :, t * 128 : (t + 1) * 128],
            s8_sb[:, :, w * W + c * MM_N : w * W + (c + 1) * MM_N],
            start=True,
            stop=True,
            perf_mode=DR,
        )


def _dve_pieces(cN):
    """Split the DVE range [A_ACT, M) at PSUM bank (512) boundaries and at
    cN (the negative->positive boundary), so each TS piece can fire as soon
    as its bank's matmul lands and the PSUM buffer releases early.
    Returns [(col_lo, col_hi, flip)]."""
    cuts = sorted({A_ACT, M, cN} | {k * MM_N for k in range(M // MM_N) if A_ACT < k * MM_N < M})
    pieces = []
    for lo, hi in zip(cuts[:-1], cuts[1:]):
        if lo >= A_ACT and hi <= M and lo < hi:
            pieces.append((lo, hi, hi <= cN))
    return pieces


def _build_v3(cN):
    nc = bacc.Bacc(
        "TRN2",
        target_bir_lowering=False,
        debug=False,
        enable_asserts=False,
        num_devices=N_CORES,
    )
    Exp = mybir.ActivationFunctionType.Exp
    mult = mybir.AluOpType.mult
    add = mybir.AluOpType.add
    subtract = mybir.AluOpType.subtract
    pieces = _dve_pieces(cN)

    with tile.TileContext(nc) as tc:
        with (
            tc.tile_pool(name="const", bufs=1) as cpool,
            tc.tile_pool(name="acc", bufs=4) as apool,
            tc.tile_pool(name="stg", bufs=4) as spool,
            tc.tile_pool(name="fin", bufs=4) as fpool,
            tc.tile_pool(name="psum", bufs=2, space="PSUM") as ppool,
        ):
            x8_sb, s8_sb, u_sb, out_d = _build_common(nc, tc, cpool)
            outT_sb = cpool.tile([128, N_TILES], F32)
            dvout = cpool.tile([128, D_DVE], FP16)
            # ACT exp output goes to a throwaway SBUF buffer (nobody reads
            # it): an in-place PSUM write would create a false write-vs-read
            # ordering against the DVE's read of the same PSUM tile.
            trash = cpool.tile([128, W], FP16)

            # CR pieces: split the code reduce at window edges so each part
            # can fire as soon as its TS codes land (packs DVE idle gaps).
            cr_parts = []
            for wb in range(NW):
                lo = max(A_ACT, wb * W) - A_ACT
                hi = (wb + 1) * W - A_ACT
                if hi > lo >= 0:
                    cr_parts.append((lo, hi))

            for t in range(N_TILES):
                acc = apool.tile([128, 6], F32, tag="acc")
                stg = spool.tile([128, D_DVE], U16, tag="stg")
                ncr = 0
                for w in range(NW):
                    ps_tile = ppool.tile([128, W], F32, tag="E")
                    _mm_window(nc, t, ps_tile, w, x8_sb, s8_sb)
                    wlo, whi = w * W, (w + 1) * W
                    # ACT part of this window
                    alo, ahi = wlo, min(whi, A_ACT)
                    if alo < ahi:
                        if ahi <= P_ACT:
                            acol = acc[:, 0:1]      # positives
                        elif alo < P_ACT:
                            raise AssertionError("P/N boundary must align to window edge")
                        else:
                            acol = acc[:, 1:2] if w == 1 else acc[:, 2:3]
                        nc.scalar.activation(
                            trash[:, 0 : ahi - alo],
                            ps_tile[:, alo - wlo : ahi - wlo],
                            Exp,
                            accum_out=acol,
                        )
                    # DVE (Schraudolph) part of this window
                    for plo, phi, flip in pieces:
                        if plo >= whi or phi <= wlo:
                            continue
                        nc.vector.tensor_scalar(
                            stg[:, plo - A_ACT : phi - A_ACT],
                            ps_tile[:, plo - wlo : phi - wlo],
                            A_SC,
                            B_SC + (32768.0 if flip else 0.0),
                            mult,
                            add,
                        )
                    # signed, 2^8-scaled partial sums of finished code spans.
                    # Col layout [P, N1, N2, dve1, pad0, dve2]: pairwise
                    # (even - odd) then f0[0]-f0[1]-f0[2] yields
                    # P - N1 - N2 + dve1 + dve2.
                    while ncr < len(cr_parts) and cr_parts[ncr][1] <= whi - A_ACT:
                        lo, hi = cr_parts[ncr]
                        col = (3, 5)[ncr]
                        nc.vector.tensor_scalar(
                            dvout[:, lo:hi],
                            stg[:, lo:hi].bitcast(FP16),
                            CR_SCALE,
                            0.0,
                            mult,
                            add,
                            accum_out=acc[:, col : col + 1],
                        )
                        ncr += 1
                assert ncr == 2, ncr
                nc.gpsimd.memset(acc[:, 4:5], 0.0)
                # out = u * ((P-N1) - (N2-dve1) - (0?-dve2)) pattern:
                # f0 = [P-N1, N2-dve1, pad-dve2]; out = u*(f0[0]-f0[1]-f0[2])
                f0 = fpool.tile([128, 3], F32, tag="fin")
                nc.gpsimd.tensor_tensor(f0[:], acc[:, 0:6:2], acc[:, 1:6:2], subtract)
                f1 = fpool.tile([128, 1], F32, tag="fin2")
                nc.gpsimd.tensor_tensor(f1[:], f0[:, 0:1], f0[:, 1:2], subtract)
                f2 = fpool.tile([128, 1], F32, tag="fin3")
                nc.gpsimd.tensor_tensor(f2[:], f1[:], f0[:, 2:3], subtract)
                nc.gpsimd.tensor_tensor(
                    outT_sb[:, t : t + 1], f2[:], u_sb[:, t : t + 1], mult
                )

            nc.sync.dma_start(out_d.ap()[:], outT_sb[:])

    nc.compile()
    return nc


def _f8(v):
    return v.astype(f8)


def _prepare(x, supports, alphas):
    x = np.asarray(x, dtype=np.float32)
    supports = np.asarray(supports, dtype=np.float32)
    alphas = np.asarray(alphas, dtype=np.float32)

    a64 = alphas.astype(np.float64)
    s64 = supports.astype(np.float64)
    jterm = -GAMMA * (s64 * s64).sum(axis=1) + np.maximum(
        np.log(np.maximum(np.abs(a64), 1e-300)), -11.0
    )

    order = np.argsort(np.abs(a64), kind="stable")
    allP = order[a64[order] > 0]
    allN = order[a64[order] <= 0]
    n_act_N = A_ACT - P_ACT
    assert len(allP) >= P_ACT and len(allN) >= n_act_N, (len(allP), len(allN))
    act_P = allP[-P_ACT:]
    dve_P = allP[:-P_ACT]
    act_N = allN[-n_act_N:]
    dve_N = allN[:-n_act_N]
    perm = np.concatenate([act_P, act_N, dve_N, dve_P])
    cN = A_ACT + len(dve_N)  # DVE cols in [A_ACT, cN) are negative-alpha

    # fp8 range-scaled hi/lo splits
    xs4 = (x.T / 4.0).astype(np.float64)              # [F, N]
    sp8 = (supports[perm].T / 8.0).astype(np.float64)  # [F, M]
    A1 = _f8(xs4)
    A2 = _f8(16.0 * (xs4 - A1.astype(np.float64)))
    A1o16 = _f8(A1.astype(np.float64) / 16.0)
    B1 = _f8(sp8)
    B2 = _f8(16.0 * (sp8 - B1.astype(np.float64)))
    B1o16 = _f8(B1.astype(np.float64) / 16.0)
    jt = jterm[perm]
    J1 = _f8(jt)
    J2 = _f8(16.0 * (jt - J1.astype(np.float64)))
    J3 = _f8(256.0 * (jt - J1.astype(np.float64) - J2.astype(np.float64) / 16.0))

    # pack logical rows r -> (p = r//2, i = r%2)
    xrows = np.zeros((256, N), dtype=f8)
    srows = np.zeros((256, M), dtype=f8)
    xrows[0:64] = A1
    srows[0:64] = B1
    xrows[64:128] = A1o16
    srows[64:128] = B2
    xrows[128:192] = A2
    srows[128:192] = B1o16
    xrows[192] = f8(1.0)
    srows[192] = J1
    xrows[193] = f8(0.0625)
    srows[193] = J2
    xrows[194] = f8(0.00390625)
    srows[194] = J3
    x8 = xrows.reshape(128, 2, N)
    s8 = srows.reshape(128, 2, M)

    u = np.exp(-GAMMA * (x.astype(np.float64) ** 2).sum(axis=1)).astype(np.float32)

    in_maps = []
    for c in range(N_CORES):
        sl = slice(c * N_LOC, (c + 1) * N_LOC)
        in_maps.append(
            {
                "x8": np.ascontiguousarray(x8[:, :, sl]),
                "s8": s8,
                "u": np.ascontiguousarray(u[sl].reshape(N_TILES, 128).T),
            }
        )
    return cN, in_maps


def _run(x, supports, alphas, trace=False, **run_kwargs):
    cN, in_maps = _prepare(x, supports, alphas)
    key = (cN, A_ACT, SIGMA)
    if key not in _compiled_cache:
        _compiled_cache[key] = _build_v3(cN)
    nc = _compiled_cache[key]
    res = run_bass_kernel_spmd(
        nc, in_maps, core_ids=list(range(N_CORES)), trace=trace, **run_kwargs
    )
    outs = [r["out"].T.reshape(-1) for r in res.results]
    return np.concatenate(outs).astype(np.float32), res


def kernel(x, supports, alphas):
    out, _ = _run(x, supports, alphas, trace=False)
    return out


# revision 10
# speedup vs baseline: 1.0571x; 1.0134x over previous
"""RBF-kernel SVM decision function on 8 TRN2 NeuronCores.

out[i] = sum_j alphas[j] * exp(-GAMMA * ||x[i] - supports[j]||^2)

Factorization: out_i = u_i * sum_j sgn_j * exp(e_ij),
  e_ij = (x_i/32).s_j + (ln|a_j| - g|s_j|^2)   [PSUM via one fp8 DoubleRow
         matmul; see row scheme below]
  u_i  = exp(-g|x_i|^2)                        [applied at the end, per i]

PE: bf16 matmuls stream 512-col chunks at ~427ns on this silicon; fp8
streams at ~216ns. To keep fp8's speed without its 3-bit-mantissa noise,
the exponent is built from range-scaled e4m3 hi/lo splits contracted in
one DoubleRow matmul ([128 partitions, 2 sub-rows], 196 live rows):
  A1=e4m3(x/4), A2=e4m3(16(x/4-A1)); B1=e4m3(s/8), B2=e4m3(16(s/8-B1))
  w = A1.B1 + (A1/16).B2 + A2.(B1/16)      (al*bl term dropped)
  jterm via 3 rows: J1 + J2/16 + J3/256
Measured exponent error ~6e-4 rms -- negligible vs the DVE approximation.

Both ScalarE (ACT) and the DVE drain PSUM in parallel, each doing the
complete exp+reduce job for its own j-range:
  - ACT, j in [0, A_ACT): native ACTIVATE(Exp, accum_out) in place on
    PSUM; pieces are sign-pure by construction of the host permutation
    (2048 largest-|a| positives first, then largest negatives).
  - DVE, j in [A_ACT, M): Schraudolph exp. One TENSOR_SCALAR computes
    round(1024*(log2e*e + 23 - SIGMA)) from PSUM fp32 into a uint16
    tile; the codes reinterpreted as fp16 ARE ~exp(e)*2^8 (pw-linear
    2^t, ~1.8% rms). Negative-alpha pieces use B+32768 (sign bit), so
    one TENSOR_SCALAR_CACHE_REDUCE (scale 2^-8) over the bitcast codes
    yields the signed sum directly. The DVE range gets the smallest-|a|
    supports so its error is weighted by the least-important columns.
Final per-tile combine (accP - accN1 - accN2 + dveS) * u runs on the
otherwise idle Pool (GPSIMD) engine.
"""

import os
import sys

for p in ("/opt/trn_rl_repo",):
    if p not in sys.path:
        sys.path.insert(0, p)

import numpy as np
import ml_dtypes

import concourse.bass as bass
import concourse.tile as tile
from concourse import bacc, mybir
from concourse.bass_utils import run_bass_kernel_spmd

N_CORES = 8
N = 16384
M = 8192
F = 64
GAMMA = 1.0 / F
N_LOC = N // N_CORES        # 2048 queries per core
N_TILES = N_LOC // 128      # 16 i-tiles of 128 queries
W = 2048                    # j-window: 4 PSUM banks
NW = M // W                 # 4 windows per j sweep
MM_N = 512                  # matmul moving free dim (1 PSUM bank)
NROW = 196                  # live contraction rows (of 256)

A_ACT = 5760                # ACT engine j-columns per sweep (rest -> DVE)
D_DVE = M - A_ACT
P_ACT = 2048                # positives in ACT range (= window 0 exactly)

# Schraudolph constants (fp16 code format): v = round(1024*(log2e*e + C -
# SIGMA)); the uint16 pattern read as fp16 is ~exp(e)*2^(C-15). SIGMA
# tuned for zero mean error under round-to-nearest (HW-verified rint).
SIGMA = float(os.environ.get("BASS_SIGMA", "0.0575"))
C16 = 23.0
A_SC = 1024.0 * np.log2(np.e)
B_SC = 1024.0 * (C16 - SIGMA)
CR_SCALE = 2.0 ** (15 - C16)

BF16 = mybir.dt.bfloat16
FP16 = mybir.dt.float16
F32 = mybir.dt.float32
U16 = mybir.dt.uint16
FP8 = mybir.dt.float8e4
bf16 = ml_dtypes.bfloat16
f8 = ml_dtypes.float8_e4m3fn

_compiled_cache = {}


def _build_common(nc, tc, cpool):
    """Input DRAM tensors, exp-table warmer, and input DMAs ordered so the
    first windows' operands land first."""
    x8_d = nc.dram_tensor("x8", [128, 2, N_LOC], FP8, kind="ExternalInput")
    s8_d = nc.dram_tensor("s8", [128, 2, M], FP8, kind="ExternalInput")
    u_d = nc.dram_tensor("u", [128, N_TILES], F32, kind="ExternalInput")
    out_d = nc.dram_tensor("out", [128, N_TILES], F32, kind="ExternalOutput")

    warm_act = cpool.tile([128, 1], F32)
    nc.gpsimd.memset(warm_act[:], 0.0)
    nc.scalar.activation(warm_act[:], warm_act[:], mybir.ActivationFunctionType.Exp)

    s8_sb = cpool.tile([128, 2, M], FP8)
    nc.sync.dma_start(s8_sb[:, :, 0:W], s8_d.ap()[:, :, 0:W])
    x8_sb = cpool.tile([128, 2, N_LOC], FP8)
    nc.sync.dma_start(x8_sb[:, :, 0:128], x8_d.ap()[:, :, 0:128])
    u_sb = cpool.tile([128, N_TILES], F32)
    nc.sync.dma_start(u_sb[:], u_d.ap()[:])
    for w in range(1, NW):
        nc.sync.dma_start(
            s8_sb[:, :, w * W : (w + 1) * W],
            s8_d.ap()[:, :, w * W : (w + 1) * W],
        )
    nc.sync.dma_start(x8_sb[:, :, 128:], x8_d.ap()[:, :, 128:])
    return x8_sb, s8_sb, u_sb, out_d


def _mm_window(nc, t, ps_tile, w, x8_sb, s8_sb):
    DR = mybir.MatmulPerfMode.DoubleRow
    for c in range(W // MM_N):
        nc.tensor.matmul(
            ps_tile[:, c * MM_N : (c + 1) * MM_N],
            x8_sb[:, :, t * 128 : (t + 1) * 128],
            s8_sb[:, :, w * W + c * MM_N : w * W + (c + 1) * MM_N],
            start=True,
            stop=True,
            perf_mode=DR,
        )


def _dve_pieces(cN):
    """Split the DVE range [A_ACT, M) at PSUM bank (512) boundaries and at
    cN (the negative->positive boundary), so each TS piece can fire as soon
    as its bank's matmul lands and the PSUM buffer releases early.
    Returns [(col_lo, col_hi, flip)]."""
    cuts = sorted({A_ACT, M, cN} | {k * MM_N for k in range(M // MM_N) if A_ACT < k * MM_N < M})
    pieces = []
    for lo, hi in zip(cuts[:-1], cuts[1:]):
        if lo >= A_ACT and hi <= M and lo < hi:
            pieces.append((lo, hi, hi <= cN))
    return pieces


def _build_v3(cN):
    nc = bacc.Bacc(
        "TRN2",
        target_bir_lowering=False,
        debug=False,
        enable_asserts=False,
        num_devices=N_CORES,
    )
    Exp = mybir.ActivationFunctionType.Exp
    mult = mybir.AluOpType.mult
    add = mybir.AluOpType.add
    subtract = mybir.AluOpType.subtract
    pieces = _dve_pieces(cN)

    with tile.TileContext(nc) as tc:
        with (
            tc.tile_pool(name="const", bufs=1) as cpool,
            tc.tile_pool(name="acc", bufs=8) as apool,
            tc.tile_pool(name="stg", bufs=4) as spool,
            tc.tile_pool(name="fin", bufs=8) as fpool,
            tc.tile_pool(name="tree", bufs=2) as tpool,
            tc.tile_pool(name="psum", bufs=2, space="PSUM") as ppool,
        ):
            x8_sb, s8_sb, u_sb, out_d = _build_common(nc, tc, cpool)
            outT_sb = cpool.tile([128, N_TILES], F32)
            dvout = cpool.tile([128, D_DVE], FP16)
            # ACT exp output goes to a throwaway SBUF buffer (nobody reads
            # it): an in-place PSUM write would create a false write-vs-read
            # ordering against the DVE's read of the same PSUM tile.
            trash = cpool.tile([128, W], FP16)

            for t in range(N_TILES):
                acc = apool.tile([128, 4], F32, tag="acc")
                stg = spool.tile([128, D_DVE], U16, tag="stg")
                for w in range(NW):
                    ps_tile = ppool.tile([128, W], F32, tag="E")
                    _mm_window(nc, t, ps_tile, w, x8_sb, s8_sb)
                    wlo, whi = w * W, (w + 1) * W
                    # ACT part of this window
                    alo, ahi = wlo, min(whi, A_ACT)
                    if alo < ahi:
                        if ahi <= P_ACT:
                            acol = acc[:, 0:1]      # positives
                        elif alo < P_ACT:
                            raise AssertionError("P/N boundary must align to window edge")
                        else:
                            acol = acc[:, 1:2] if w == 1 else acc[:, 2:3]
                        nc.scalar.activation(
                            trash[:, 0 : ahi - alo],
                            ps_tile[:, alo - wlo : ahi - wlo],
                            Exp,
                            accum_out=acol,
                        )
                    # DVE (Schraudolph) part of this window
                    for plo, phi, flip in pieces:
                        if plo >= whi or phi <= wlo:
                            continue
                        nc.vector.tensor_scalar(
                            stg[:, plo - A_ACT : phi - A_ACT],
                            ps_tile[:, plo - wlo : phi - wlo],
                            A_SC,
                            B_SC + (32768.0 if flip else 0.0),
                            mult,
                            add,
                        )
                # The code reduce runs as a halving tree on the otherwise
                # idle Pool engine (contiguous fp16 adds), so the DVE's only
                # remaining reduce is one tiny CACHE_REDUCE. This keeps big
                # DVE ops out of the PE's PSUM-release chain.
                h1, h2, h3 = D_DVE // 2, D_DVE // 4, D_DVE // 8
                t1 = tpool.tile([128, h1], FP16, tag="t1")
                nc.gpsimd.tensor_tensor(
                    t1[:], stg[:, 0:h1].bitcast(FP16), stg[:, h1:].bitcast(FP16), add
                )
                t2 = tpool.tile([128, h2], FP16, tag="t2")
                nc.gpsimd.tensor_tensor(t2[:], t1[:, 0:h2], t1[:, h2:], add)
                t3 = tpool.tile([128, h3], FP16, tag="t3")
                nc.gpsimd.tensor_tensor(t3[:], t2[:, 0:h3], t2[:, h3:], add)
                nc.vector.tensor_scalar(
                    dvout[:, 0:h3],
                    t3[:],
                    CR_SCALE,
                    0.0,
                    mult,
                    add,
                    accum_out=acc[:, 3:4],
                )
                # out = u * ((P - N1) - (N2 - dve))
                f0 = fpool.tile([128, 2], F32, tag="fin")
                nc.gpsimd.tensor_tensor(f0[:], acc[:, 0:4:2], acc[:, 1:4:2], subtract)
                f1 = fpool.tile([128, 1], F32, tag="fin2")
                nc.gpsimd.tensor_tensor(f1[:], f0[:, 0:1], f0[:, 1:2], subtract)
                nc.gpsimd.tensor_tensor(
                    outT_sb[:, t : t + 1], f1[:], u_sb[:, t : t + 1], mult
                )

            nc.sync.dma_start(out_d.ap()[:], outT_sb[:])

    nc.compile()
    return nc


def _f8(v):
    return v.astype(f8)


def _prepare(x, supports, alphas):
    x = np.asarray(x, dtype=np.float32)
    supports = np.asarray(supports, dtype=np.float32)
    alphas = np.asarray(alphas, dtype=np.float32)

    a64 = alphas.astype(np.float64)
    s64 = supports.astype(np.float64)
    jterm = -GAMMA * (s64 * s64).sum(axis=1) + np.maximum(
        np.log(np.maximum(np.abs(a64), 1e-300)), -11.0
    )

    order = np.argsort(np.abs(a64), kind="stable")
    allP = order[a64[order] > 0]
    allN = order[a64[order] <= 0]
    n_act_N = A_ACT - P_ACT
    assert len(allP) >= P_ACT and len(allN) >= n_act_N, (len(allP), len(allN))
    act_P = allP[-P_ACT:]
    dve_P = allP[:-P_ACT]
    act_N = allN[-n_act_N:]
    dve_N = allN[:-n_act_N]
    perm = np.concatenate([act_P, act_N, dve_N, dve_P])
    cN = A_ACT + len(dve_N)  # DVE cols in [A_ACT, cN) are negative-alpha

    # fp8 range-scaled hi/lo splits
    xs4 = (x.T / 4.0).astype(np.float64)              # [F, N]
    sp8 = (supports[perm].T / 8.0).astype(np.float64)  # [F, M]
    A1 = _f8(xs4)
    A2 = _f8(16.0 * (xs4 - A1.astype(np.float64)))
    A1o16 = _f8(A1.astype(np.float64) / 16.0)
    B1 = _f8(sp8)
    B2 = _f8(16.0 * (sp8 - B1.astype(np.float64)))
    B1o16 = _f8(B1.astype(np.float64) / 16.0)
    jt = jterm[perm]
    J1 = _f8(jt)
    J2 = _f8(16.0 * (jt - J1.astype(np.float64)))
    J3 = _f8(256.0 * (jt - J1.astype(np.float64) - J2.astype(np.float64) / 16.0))

    # pack logical rows r -> (p = r//2, i = r%2)
    xrows = np.zeros((256, N), dtype=f8)
    srows = np.zeros((256, M), dtype=f8)
    xrows[0:64] = A1
    srows[0:64] = B1
    xrows[64:128] = A1o16
    srows[64:128] = B2
    xrows[128:192] = A2
    srows[128:192] = B1o16
    xrows[192] = f8(1.0)
    srows[192] = J1
    xrows[193] = f8(0.0625)
    srows[193] = J2
    xrows[194] = f8(0.00390625)
    srows[194] = J3
    x8 = xrows.reshape(128, 2, N)
    s8 = srows.reshape(128, 2, M)

    u = np.exp(-GAMMA * (x.astype(np.float64) ** 2).sum(axis=1)).astype(np.float32)

    in_maps = []
    for c in range(N_CORES):
        sl = slice(c * N_LOC, (c + 1) * N_LOC)
        in_maps.append(
            {
                "x8": np.ascontiguousarray(x8[:, :, sl]),
                "s8": s8,
                "u": np.ascontiguousarray(u[sl].reshape(N_TILES, 128).T),
            }
        )
    return cN, in_maps


def _run(x, supports, alphas, trace=False, **run_kwargs):
    cN, in_maps = _prepare(x, supports, alphas)
    key = (cN, A_ACT, SIGMA)
    if key not in _compiled_cache:
        _compiled_cache[key] = _build_v3(cN)
    nc = _compiled_cache[key]
    res = run_bass_kernel_spmd(
        nc, in_maps, core_ids=list(range(N_CORES)), trace=trace, **run_kwargs
    )
    outs = [r["out"].T.reshape(-1) for r in res.results]
    return np.concatenate(outs).astype(np.float32), res


def kernel(x, supports, alphas):
    out, _ = _run(x, supports, alphas, trace=False)
    return out
